# revision 3
# baseline (speedup 1.0000x reference)
"""Trainium2 Bass kernel for the dynamic segment-aggregation module.

Computation per (clip n, channel c):
  pooled[u]  = mean_{t,h,w} x[n,c,u,...]                (U=4 segments)
  z          = relu(BN(pooled @ W1^T))                  (tiny MLP, eval-mode BN)
  kern       = softmax(z @ W2^T)                        (K=3 taps)
  out[u]     = kern[0]*x[u-1] + kern[1]*x[u] + kern[2]*x[u+1]   (zero-padded)

Sharding: data-parallel over the 8 clips -> 1 clip (4 U-segments) per
NeuronCore; the tiny generator weights are replicated (packed into one
64-float tensor, BN scale folded into W1, BN offset and the 1/THW pooling
mean folded host-side).

I/O staging is bf16: the host converts x to bf16 in a [C, NQ, U, FQ]
layout (each 128-channel slab load/store is one 12.5KB-contiguous
descriptor per partition) and converts the bf16 output back. This halves
HBM traffic (51.4 -> 25.7 MB/core, ~72us at 358 GB/s) and puts the DVE
in its 2x/4x packed 16-bit modes (bf16 only; IEEE fp16 ran 1x). bf16 keeps rel err ~3e-3,
well inside the 2e-2 gate.

Per-core schedule:
  - channels on the 128 SBUF partitions (2 groups of 128), free dim =
    (u, t-quarter*h*w) so a segment shift of +-1 is just a +-FQ column
    offset into the same slab.
  - taps per slab run as 4 whole-slab passes instead of 10 per-u passes:
      P1 (ScalarE): oa[0:3F] = k2 * x[F:4F]
      P2 (DVE 4x):  oa[3F:4F] = k1 * x[3F:4F]
      P3 (DVE 2x):  oa[0:3F] += k1 * x[0:3F]
      P4 (DVE 2x):  oa[F:4F] += k0 * x[0:3F]
  - group-0 pools ride ScalarE's activation accumulator (u0-2) + one DVE
    tensor_scalar accumulator (u3) so they keep pace with the loads;
    group-1 pools are all DVE tensor_scalar (4x mode), interleaved
    between group-0 tap quarters so the MLP for group 1 is ready the
    moment group-0 taps drain.
  - loads on the Sync HWDGE queue (all 8 slabs queued up front, every
    slab has its own buffer), stores on the GpSimd SWDGE queue with
    explicit deps that yield HBM to the remaining loads.
"""

import numpy as np
import ml_dtypes

import concourse.bass as bass
import concourse.bacc as bacc
import concourse.tile as tile
from concourse import mybir
from concourse.bass_utils import run_bass_kernel_spmd

U = 4          # segments per clip
C = 256        # channels
T, H, W = 8, 28, 28
THW = T * H * W            # 6272
NQ = 4                     # t-quarters per channel-group
FQ = THW // NQ             # 1568
UF = U * FQ                # 6272 (flat slab free dim)
D = 8                      # MLP hidden dim (U * alpha)
K = 3                      # conv taps
EPS = 1e-5
N_CORES = 8
NCG = C // 128             # channel groups per core

# packed small-weights layout: [W1*s/THW (D*U) | W2 (K*D) | t (D)]
NPACK = D * U + K * D + D    # 64

FP32 = mybir.dt.float32
BF16 = mybir.dt.bfloat16

_nc_cache = None
last_results = None        # BassKernelResults of the most recent run (for test.py)


def _bcast_ap(ap, parts=128):
    """DRAM AP replicated across `parts` partitions (partition stride 0)."""
    return bass.AP(tensor=ap.tensor, offset=ap.offset, ap=[[0, parts]] + list(ap.ap))


def _build_nc():
    nc = bacc.Bacc(None, target_bir_lowering=False)
    x_h = nc.declare_dram_parameter("x", [C, NQ * UF], BF16, isOutput=False)
    wp_h = nc.declare_dram_parameter("wpack", [NPACK], FP32, isOutput=False)
    out_h = nc.declare_dram_parameter("out", [C, NQ * UF], BF16, isOutput=True)

    AX = mybir.AxisListType
    OP = mybir.AluOpType
    AF = mybir.ActivationFunctionType

    with tile.TileContext(nc) as tc:
        with (
            tc.tile_pool(name="xp", bufs=8) as xp,
            tc.tile_pool(name="outp", bufs=6) as outp,
            tc.tile_pool(name="small", bufs=1) as small,
            tc.tile_pool(name="mlp", bufs=2) as mlp,
        ):
            # one tiny DMA for every per-core-replicated constant
            wpk = small.tile([128, NPACK], FP32)
            nc.gpsimd.dma_start(out=wpk, in_=_bcast_ap(wp_h[:]))
            w1sb = wpk[:, 0:D * U].rearrange("p (d u) -> p d u", d=D)       # [128,D,U]
            w2sb = wpk[:, D * U:D * U + K * D].rearrange(
                "p (k d) -> p k d", k=K)                                    # [128,K,D]
            o_t = wpk[:, D * U + K * D:NPACK]                               # [128,D]

            def load_slab(g, q):
                c0 = g * 128
                sl = xp.tile([128, UF], BF16, tag="xslab")
                ld = nc.sync.dma_start(
                    out=sl, in_=x_h[c0:c0 + 128, q * UF:(q + 1) * UF]
                )
                return sl, ld

            def pool_scalar(sl, q, P, us):
                """ScalarE activation accumulator (identity copy in place)."""
                su = sl.rearrange("p (u f) -> p u f", u=U)
                for u in us:
                    nc.scalar.activation(
                        out=su[:, u, :], in_=su[:, u, :], func=AF.Copy,
                        accum_out=P[:, u, q:q + 1],
                    )

            def pool_vector(sl, q, P, us):
                """DVE tensor_scalar accumulator (4x packed fp16 mode)."""
                su = sl.rearrange("p (u f) -> p u f", u=U)
                for u in us:
                    nc.vector.tensor_scalar(
                        out=su[:, u, :], in0=su[:, u, :], scalar1=1.0,
                        scalar2=0.0, op0=OP.mult, op1=OP.add,
                        accum_out=P[:, u, q:q + 1],
                    )

            def gen_mlp(P):
                """pooled -> relu(BN(pooled@W1s^T)) -> softmax(z@W2^T)."""
                pooled = mlp.tile([128, U], FP32, tag="pooled")
                nc.vector.reduce_sum(out=pooled, in_=P, axis=AX.X)
                z = mlp.tile([128, D], FP32, tag="z")
                nc.vector.tensor_scalar_mul(
                    out=z, in0=w1sb[:, :, 0], scalar1=pooled[:, 0:1]
                )
                for u in range(1, U):
                    nc.vector.scalar_tensor_tensor(
                        out=z, in0=w1sb[:, :, u], scalar=pooled[:, u:u + 1],
                        in1=z, op0=OP.mult, op1=OP.add,
                    )
                nc.vector.tensor_add(out=z, in0=z, in1=o_t)
                nc.vector.tensor_scalar_max(out=z, in0=z, scalar1=0.0)
                logit = mlp.tile([128, K], FP32, tag="logit")
                nc.vector.tensor_scalar_mul(
                    out=logit, in0=w2sb[:, :, 0], scalar1=z[:, 0:1]
                )
                for d in range(1, D):
                    nc.vector.scalar_tensor_tensor(
                        out=logit, in0=w2sb[:, :, d], scalar=z[:, d:d + 1],
                        in1=logit, op0=OP.mult, op1=OP.add,
                    )
                mx = mlp.tile([128, 1], FP32, tag="mx")
                nc.vector.reduce_max(out=mx, in_=logit, axis=AX.X)
                nc.vector.tensor_scalar_mul(out=mx, in0=mx, scalar1=-1.0)
                nc.scalar.activation(
                    out=logit, in_=logit, func=AF.Exp, bias=mx[:, 0:1]
                )
                ssum = mlp.tile([128, 1], FP32, tag="ssum")
                nc.vector.reduce_sum(out=ssum, in_=logit, axis=AX.X)
                nc.vector.reciprocal(out=ssum, in_=ssum)
                kern = mlp.tile([128, K], FP32, tag="kern")
                nc.vector.tensor_scalar_mul(out=kern, in0=logit, scalar1=ssum[:, 0:1])
                return kern

            F3 = 3 * FQ

            def tap_scale(g, q, sl, kern):
                """P1 on ScalarE: oa[0:3F] = k2 * x[F:4F]."""
                oa = outp.tile([128, UF], BF16, tag="outslab")
                nc.scalar.activation(
                    out=oa[:, 0:F3], in_=sl[:, FQ:UF], func=AF.Copy,
                    scale=kern[:, 2:3],
                )
                return oa

            def tap_rest(g, q, sl, oa, kern):
                """P2..P4 on DVE, then store on the GpSimd SWDGE queue."""
                c0 = g * 128
                k0, k1 = kern[:, 0:1], kern[:, 1:2]
                nc.vector.tensor_scalar_mul(
                    out=oa[:, F3:UF], in0=sl[:, F3:UF], scalar1=k1
                )
                nc.vector.scalar_tensor_tensor(
                    out=oa[:, 0:F3], in0=sl[:, 0:F3], scalar=k1,
                    in1=oa[:, 0:F3], op0=OP.mult, op1=OP.add,
                )
                nc.vector.scalar_tensor_tensor(
                    out=oa[:, FQ:UF], in0=sl[:, 0:F3], scalar=k0,
                    in1=oa[:, FQ:UF], op0=OP.mult, op1=OP.add,
                )
                st = nc.gpsimd.dma_start(
                    out=out_h[c0:c0 + 128, q * UF:(q + 1) * UF], in_=oa
                )
                return st

            # ---- schedule ----
            from concourse.tile_rust import add_dep_helper

            # all 8 slab loads queued up front on the Sync HWDGE queue
            P0 = mlp.tile([128, U, NQ], FP32, tag="P")
            P1t = mlp.tile([128, U, NQ], FP32, tag="P")
            g0 = [load_slab(0, q) for q in range(NQ)]
            g1 = [load_slab(1, q) for q in range(NQ)]

            # group-0 pools keep pace with the loads: u0-2 on ScalarE's
            # activation accumulator, u3 on the DVE accumulator
            for q in range(NQ):
                pool_scalar(g0[q][0], q, P0, (0, 1, 2))
                pool_vector(g0[q][0], q, P0, (3,))
            kern0 = gen_mlp(P0)

            # group-0 taps; group-1 pools (all DVE, 4x) interleave between
            # tap quarters so kern1 is ready when group-0 taps drain
            oas0 = [tap_scale(0, q, g0[q][0], kern0) for q in range(NQ)]
            for q in range(NQ):
                st = tap_rest(0, q, g0[q][0], oas0[q], kern0)
                pool_vector(g1[q][0], q, P1t, (0, 1, 2, 3))
                # defer group-0 stores behind the matching group-1 load so
                # loads keep HBM priority
                add_dep_helper(st.ins, g1[q][1].ins,
                               reason="store yields HBM to next load")
            kern1 = gen_mlp(P1t)

            oas1 = [tap_scale(1, q, g1[q][0], kern1) for q in range(NQ)]
            for q in range(NQ):
                tap_rest(1, q, g1[q][0], oas1[q], kern1)
    nc.finalize()
    return nc


def _get_nc():
    global _nc_cache
    if _nc_cache is None:
        _nc_cache = _build_nc()
    return _nc_cache


def _pack_small(W1, bn_gamma, bn_beta, bn_mean, bn_var, W2):
    W1 = np.asarray(W1, np.float64)
    W2 = np.asarray(W2, np.float32)
    gam = np.asarray(bn_gamma, np.float64)
    bet = np.asarray(bn_beta, np.float64)
    mea = np.asarray(bn_mean, np.float64)
    var = np.asarray(bn_var, np.float64)
    s = gam / np.sqrt(var + np.float64(EPS))
    t = (bet - mea * s).astype(np.float32)
    w1s = (W1 * s[:, None] * (1.0 / THW)).astype(np.float32)
    return np.concatenate(
        [w1s.reshape(-1), W2.reshape(-1), t]
    ).astype(np.float32)


def _ensure_hook_stub():
    """bass_utils' trace path imports antenv.axon_hooks when BASS_TRACE is
    set; if this image lacks it, register a None-returning stub so the run
    degrades to no-trace instead of crashing."""
    import sys
    import types

    try:
        import antenv.axon_hooks  # noqa: F401
    except ImportError:
        mod = types.ModuleType("antenv.axon_hooks")
        mod.get_axon_ntff_profile_hook = lambda: None
        mod.set_axon_ntff_profile_hook = lambda h: None
        sys.modules["antenv.axon_hooks"] = mod


def kernel(x, W1, bn_gamma, bn_beta, bn_mean, bn_var, W2):
    global last_results
    _ensure_hook_stub()
    nc = _get_nc()
    # host-side staging: fp16, [C, NQ, U, FQ] per core so every slab DMA is
    # one 12.5KB-contiguous run per partition
    x = np.asarray(x, dtype=np.float32).reshape(N_CORES, U, C, NQ, FQ)
    xdev = x.transpose(0, 2, 3, 1, 4).astype(ml_dtypes.bfloat16)  # [8, C, NQ, U, FQ]
    wpack = _pack_small(W1, bn_gamma, bn_beta, bn_mean, bn_var, W2)
    in_maps = [
        {"x": np.ascontiguousarray(xdev[i]).reshape(C, NQ * UF), "wpack": wpack}
        for i in range(N_CORES)
    ]
    last_results = run_bass_kernel_spmd(nc, in_maps, list(range(N_CORES)))
    out = np.stack([last_results.results[i]["out"] for i in range(N_CORES)])
    out = out.reshape(N_CORES, C, NQ, U, FQ).transpose(0, 3, 1, 2, 4)
    return np.ascontiguousarray(out).astype(np.float32).reshape(
        N_CORES * U, C, T, H, W
    )


# revision 4
# speedup vs baseline: 1.3656x; 1.3656x over previous
"""Trainium2 Bass kernel for the dynamic segment-aggregation module.

Computation per (clip n, channel c):
  pooled[u]  = mean_{t,h,w} x[n,c,u,...]                (U=4 segments)
  z          = relu(BN(pooled @ W1^T))                  (tiny MLP, eval-mode BN)
  kern       = softmax(z @ W2^T)                        (K=3 taps)
  out[u]     = kern[0]*x[u-1] + kern[1]*x[u] + kern[2]*x[u+1]   (zero-padded)

Sharding: data-parallel over the 8 clips -> 1 clip per NeuronCore; the tiny
generator weights are replicated (BN scale folded into W1, BN offset and the
1/THW pooling mean folded host-side).

I/O staging is bf16 in a [C, NQ, U, FQ] layout (each 128-channel slab
load/store is one 12.5KB-contiguous run per partition): halves HBM traffic
to 25.7 MB/core (~72us at 358 GB/s). bf16 keeps rel err ~7e-3, inside the
2e-2 gate.

The 3-tap blend runs on the (otherwise idle) TensorEngine: per tap j a
diagonal 128x128 stationary matrix diag(kern[:, j]) scales all 128 channels
at once (built in one 4x-mode tensor_scalar from a host-supplied identity),
and the K=3 taps accumulate in PSUM across matmuls:
  psum[u-wave] = diag(k1) @ x[u]  (+ diag(k0) @ x[u-1]) (+ diag(k2) @ x[u+1])
Each u-wave is one [128, FQ] PSUM region written in 512-col bank chunks,
tap-major so the stationary weights reload only 2-3x per wave. DVE/ScalarE
then downconvert PSUM fp32 -> bf16 out slab (3 copies DVE, 1 ScalarE per
slab) -- the only per-element vector work left besides pooling.

Pooling rides ScalarE's activation accumulator (group 0 puts u3 on the DVE
accumulator so pools keep pace with the loads and the group-0 MLP starts
~5us after its last load). Loads on the Sync HWDGE queue (all 8 slabs
queued up front), stores on the GpSimd SWDGE queue with explicit deps that
yield HBM to the remaining loads.
"""

import numpy as np
import ml_dtypes

import concourse.bass as bass
import concourse.bacc as bacc
import concourse.tile as tile
from concourse import mybir
from concourse.bass_utils import run_bass_kernel_spmd

U = 4          # segments per clip
C = 256        # channels
T, H, W = 8, 28, 28
THW = T * H * W            # 6272
NQ = 4                     # t-quarters per channel-group
FQ = THW // NQ             # 1568
UF = U * FQ                # 6272 (flat slab free dim)
D = 8                      # MLP hidden dim (U * alpha)
K = 3                      # conv taps
EPS = 1e-5
N_CORES = 8
BANK = 512                 # fp32 elems per PSUM bank

# packed small-weights layout: [W1*s/THW (D*U) | W2 (K*D) | t (D)]
NPACK = D * U + K * D + D    # 64

FP32 = mybir.dt.float32
BF16 = mybir.dt.bfloat16

# taps hitting segment u (tap j reads moving segment u + j - 1)
TAPS = {0: (1, 2), 1: (0, 1, 2), 2: (0, 1, 2), 3: (0, 1)}

_nc_cache = None
last_results = None        # BassKernelResults of the most recent run (for test.py)


def _bcast_ap(ap, parts=128):
    """DRAM AP replicated across `parts` partitions (partition stride 0)."""
    return bass.AP(tensor=ap.tensor, offset=ap.offset, ap=[[0, parts]] + list(ap.ap))


def _build_nc():
    nc = bacc.Bacc(None, target_bir_lowering=False)
    x_h = nc.declare_dram_parameter("x", [C, NQ * UF], BF16, isOutput=False)
    wp_h = nc.declare_dram_parameter("wpack", [NPACK], FP32, isOutput=False)
    id_h = nc.declare_dram_parameter("ident", [128, 128], BF16, isOutput=False)
    out_h = nc.declare_dram_parameter("out", [C, NQ * UF], BF16, isOutput=True)

    AX = mybir.AxisListType
    OP = mybir.AluOpType
    AF = mybir.ActivationFunctionType

    with tile.TileContext(nc) as tc:
        with (
            tc.tile_pool(name="xp", bufs=8) as xp,
            tc.tile_pool(name="outp", bufs=6) as outp,
            tc.tile_pool(name="small", bufs=1) as small,
            tc.tile_pool(name="mlp", bufs=2) as mlp,
            tc.tile_pool(name="dgp", bufs=6) as dgp,
            tc.tile_pool(name="psp", bufs=2, space="PSUM") as psp,
        ):
            # tiny DMAs for the per-core-replicated constants
            wpk = small.tile([128, NPACK], FP32)
            nc.gpsimd.dma_start(out=wpk, in_=_bcast_ap(wp_h[:]))
            id_t = small.tile([128, 128], BF16)
            nc.gpsimd.dma_start(out=id_t, in_=id_h[:])
            w1sb = wpk[:, 0:D * U].rearrange("p (d u) -> p d u", d=D)       # [128,D,U]
            w2sb = wpk[:, D * U:D * U + K * D].rearrange(
                "p (k d) -> p k d", k=K)                                    # [128,K,D]
            o_t = wpk[:, D * U + K * D:NPACK]                               # [128,D]

            def load_slab(g, q):
                c0 = g * 128
                sl = xp.tile([128, UF], BF16, tag="xslab")
                ld = nc.sync.dma_start(
                    out=sl, in_=x_h[c0:c0 + 128, q * UF:(q + 1) * UF]
                )
                return sl, ld

            def pool_scalar(sl, q, P, us):
                """ScalarE activation accumulator (identity copy in place)."""
                for u in us:
                    nc.scalar.activation(
                        out=sl[:, u * FQ:(u + 1) * FQ],
                        in_=sl[:, u * FQ:(u + 1) * FQ], func=AF.Copy,
                        accum_out=P[:, u, q:q + 1],
                    )

            def pool_vector(sl, q, P, us):
                """DVE tensor_scalar accumulator (1x, only for group-0 u3)."""
                for u in us:
                    nc.vector.tensor_scalar(
                        out=sl[:, u * FQ:(u + 1) * FQ],
                        in0=sl[:, u * FQ:(u + 1) * FQ], scalar1=1.0,
                        scalar2=0.0, op0=OP.mult, op1=OP.add,
                        accum_out=P[:, u, q:q + 1],
                    )

            def gen_mlp(P):
                """pooled -> relu(BN(pooled@W1s^T)) -> softmax(z@W2^T)."""
                pooled = mlp.tile([128, U], FP32, tag="pooled")
                nc.vector.reduce_sum(out=pooled, in_=P, axis=AX.X)
                z = mlp.tile([128, D], FP32, tag="z")
                nc.vector.tensor_scalar_mul(
                    out=z, in0=w1sb[:, :, 0], scalar1=pooled[:, 0:1]
                )
                for u in range(1, U):
                    nc.vector.scalar_tensor_tensor(
                        out=z, in0=w1sb[:, :, u], scalar=pooled[:, u:u + 1],
                        in1=z, op0=OP.mult, op1=OP.add,
                    )
                nc.vector.tensor_add(out=z, in0=z, in1=o_t)
                nc.vector.tensor_scalar_max(out=z, in0=z, scalar1=0.0)
                logit = mlp.tile([128, K], FP32, tag="logit")
                nc.vector.tensor_scalar_mul(
                    out=logit, in0=w2sb[:, :, 0], scalar1=z[:, 0:1]
                )
                for d in range(1, D):
                    nc.vector.scalar_tensor_tensor(
                        out=logit, in0=w2sb[:, :, d], scalar=z[:, d:d + 1],
                        in1=logit, op0=OP.mult, op1=OP.add,
                    )
                mx = mlp.tile([128, 1], FP32, tag="mx")
                nc.vector.reduce_max(out=mx, in_=logit, axis=AX.X)
                nc.vector.tensor_scalar_mul(out=mx, in0=mx, scalar1=-1.0)
                nc.scalar.activation(
                    out=logit, in_=logit, func=AF.Exp, bias=mx[:, 0:1]
                )
                ssum = mlp.tile([128, 1], FP32, tag="ssum")
                nc.vector.reduce_sum(out=ssum, in_=logit, axis=AX.X)
                nc.vector.reciprocal(out=ssum, in_=ssum)
                kern = mlp.tile([128, K], FP32, tag="kern")
                nc.vector.tensor_scalar_mul(out=kern, in0=logit, scalar1=ssum[:, 0:1])
                return kern

            def make_diags(kern):
                """diag(kern[:, j]) stationaries, one 4x tensor_scalar each."""
                ds = []
                for j in range(K):
                    d = dgp.tile([128, 128], BF16, tag="diag")
                    nc.vector.tensor_scalar_mul(
                        out=d, in0=id_t, scalar1=kern[:, j:j + 1]
                    )
                    ds.append(d)
                return ds

            def blend_slab(g, q, sl, diags):
                """Per u-wave: 2-3 accumulating diag-matmuls into one [128,FQ]
                PSUM region (tap-major, 512-col bank chunks), then one
                fp32->bf16 copy out (wave 0 on ScalarE, rest on DVE)."""
                c0 = g * 128
                oa = outp.tile([128, UF], BF16, tag="outslab")
                for u in range(U):
                    taps = TAPS[u]
                    Wp = psp.tile([128, 4 * BANK], FP32, tag="psum")
                    for i, j in enumerate(taps):
                        mv = sl[:, (u + j - 1) * FQ:(u + j) * FQ]
                        for cb in range(0, FQ, BANK):
                            ce = min(FQ, cb + BANK)
                            nc.tensor.matmul(
                                Wp[:, cb:ce], diags[j], mv[:, cb:ce],
                                start=(i == 0), stop=(i == len(taps) - 1),
                            )
                    dst = oa[:, u * FQ:(u + 1) * FQ]
                    if u == 0:
                        nc.scalar.activation(out=dst, in_=Wp[:, 0:FQ],
                                             func=AF.Copy)
                    else:
                        nc.vector.tensor_copy(out=dst, in_=Wp[:, 0:FQ])
                st = nc.gpsimd.dma_start(
                    out=out_h[c0:c0 + 128, q * UF:(q + 1) * UF], in_=oa
                )
                return st

            # ---- schedule ----
            from concourse.tile_rust import add_dep_helper

            P0 = mlp.tile([128, U, NQ], FP32, tag="P")
            P1t = mlp.tile([128, U, NQ], FP32, tag="P")
            g0 = [load_slab(0, q) for q in range(NQ)]
            g1 = [load_slab(1, q) for q in range(NQ)]

            # group-0 pools keep pace with the loads: u0-2 ScalarE, u3 DVE
            for q in range(NQ):
                pool_scalar(g0[q][0], q, P0, (0, 1, 2))
                pool_vector(g0[q][0], q, P0, (3,))
            kern0 = gen_mlp(P0)
            diags0 = make_diags(kern0)

            # group-0 blend; group-1 pools (all ScalarE) interleave between
            # slabs so kern1 is ready the moment the TensorE stream drains
            for q in range(NQ):
                st = blend_slab(0, q, g0[q][0], diags0)
                pool_scalar(g1[q][0], q, P1t, (0, 1, 2, 3))
                add_dep_helper(st.ins, g1[q][1].ins,
                               reason="store yields HBM to next load")
            kern1 = gen_mlp(P1t)
            diags1 = make_diags(kern1)
            for q in range(NQ):
                blend_slab(1, q, g1[q][0], diags1)
    nc.finalize()
    return nc


def _get_nc():
    global _nc_cache
    if _nc_cache is None:
        _nc_cache = _build_nc()
    return _nc_cache


def _pack_small(W1, bn_gamma, bn_beta, bn_mean, bn_var, W2):
    W1 = np.asarray(W1, np.float64)
    W2 = np.asarray(W2, np.float32)
    gam = np.asarray(bn_gamma, np.float64)
    bet = np.asarray(bn_beta, np.float64)
    mea = np.asarray(bn_mean, np.float64)
    var = np.asarray(bn_var, np.float64)
    s = gam / np.sqrt(var + np.float64(EPS))
    t = (bet - mea * s).astype(np.float32)
    w1s = (W1 * s[:, None] * (1.0 / THW)).astype(np.float32)
    return np.concatenate(
        [w1s.reshape(-1), W2.reshape(-1), t]
    ).astype(np.float32)


def _ensure_hook_stub():
    """bass_utils' trace path imports antenv.axon_hooks when BASS_TRACE is
    set; if this image lacks it, register a None-returning stub so the run
    degrades to no-trace instead of crashing."""
    import sys
    import types

    try:
        import antenv.axon_hooks  # noqa: F401
    except ImportError:
        mod = types.ModuleType("antenv.axon_hooks")
        mod.get_axon_ntff_profile_hook = lambda: None
        mod.set_axon_ntff_profile_hook = lambda h: None
        sys.modules["antenv.axon_hooks"] = mod


def kernel(x, W1, bn_gamma, bn_beta, bn_mean, bn_var, W2):
    global last_results
    _ensure_hook_stub()
    nc = _get_nc()
    # host-side staging: bf16, [C, NQ, U, FQ] per core so every slab DMA is
    # one 12.5KB-contiguous run per partition
    x = np.asarray(x, dtype=np.float32).reshape(N_CORES, U, C, NQ, FQ)
    xdev = x.transpose(0, 2, 3, 1, 4).astype(ml_dtypes.bfloat16)  # [8,C,NQ,U,FQ]
    wpack = _pack_small(W1, bn_gamma, bn_beta, bn_mean, bn_var, W2)
    ident = np.eye(128, dtype=ml_dtypes.bfloat16)
    in_maps = [
        {"x": np.ascontiguousarray(xdev[i]).reshape(C, NQ * UF),
         "wpack": wpack, "ident": ident}
        for i in range(N_CORES)
    ]
    last_results = run_bass_kernel_spmd(nc, in_maps, list(range(N_CORES)))
    out = np.stack([last_results.results[i]["out"] for i in range(N_CORES)])
    out = out.reshape(N_CORES, C, NQ, U, FQ).transpose(0, 3, 1, 2, 4)
    return np.ascontiguousarray(out).astype(np.float32).reshape(
        N_CORES * U, C, T, H, W
    )
